# revision 2
# baseline (speedup 1.0000x reference)
"""Trainium2 Bass kernel for nn_NetworkActivity_layer (masked linear):

    out = x @ (weight * mask.T).T + bias      x:(4096,15000) w:(500,15000)
                                              mask:(15000,500) bias:(500,)

Strategy: shard the contraction (gene) dim K=15000 across 8 NeuronCores
(1875 genes/core). Each core computes a partial (4096,500) fp32 output:
    partial_i = x[:, sh_i] @ (weight[:, sh_i] * mask[sh_i, :].T).T
Host sums the 8 partials (the K-shard "unshard" step). The bias is folded
into an extra padded gene row (x column of ones, weight row = bias on core
0, mask row = 1), so the device kernel computes the complete affine map.

Per-core layout (host-packed for DMA friendliness + TensorE layout):
  genes padded 1875 -> 1920 = 15 k-tiles x 128 (FWL needs K=128 exactly)
  xt: (32, 128, 1920) bf16   xt[m, p, k*128+c] = xpad[m*128+c, k*128+p]
      -> SBUF tile [128, 1920]; slice [:, k*128:(k+1)*128] is the
         stationary lhsT (K=128 genes, M=128 batch) for (m, k)
  wt/mk: (128, 7500) bf16    [p, k*500+n] = wpad/mpad[k*128+p, n]
      -> masked weights mw = wt*mk computed on-device; slice
         [:, k*500:(k+1)*500] is the moving rhs (K=128, N=500)
  out: (32, 128, 500) fp32 partial, accumulated over 15 k-tiles in PSUM.
"""

import functools
import os

import ml_dtypes
import numpy as np

B, G, P = 4096, 15000, 500
N_CORES = 8
GS = G // N_CORES          # 1875 genes per core
KT = 128                   # k-tile size (partition dim; 128 enables FWL)
NK = 15                    # k-tiles per core
KP = NK * KT               # 1920 padded genes (row GS=1875 carries bias)
MT = 128                   # batch tile
NM = B // MT               # 32 batch tiles

_BF16 = ml_dtypes.bfloat16

LAST_EXEC_TIME_NS = None
LAST_TRACE = None
LAST_RESULTS = None


def _install_profshim():
    """Make run_bass_kernel_spmd(trace=True) work in the axon container:
    recreate the antenv.axon_hooks NTFF hook + keep artifacts local."""
    import sys
    import types

    if "antenv.axon_hooks" not in sys.modules:
        import antenv
        from trn_agent_boot.trn_boot import _ntff_profile_via_ctypes

        mod = types.ModuleType("antenv.axon_hooks")
        mod._hook = _ntff_profile_via_ctypes("/opt/axon/libaxon_pjrt.so")
        mod.set_axon_ntff_profile_hook = lambda h: setattr(mod, "_hook", h)
        mod.get_axon_ntff_profile_hook = lambda: mod._hook
        sys.modules["antenv.axon_hooks"] = mod
        antenv.axon_hooks = mod

    import concourse.bass_utils as bu

    bu.upload_artifacts = lambda tmpdir: f"file://{tmpdir}"


@functools.lru_cache(maxsize=1)
def _build():
    import concourse.bass as bass
    import concourse.mybir as mybir
    import concourse.tile as tile
    from concourse import bacc

    nc = bacc.Bacc(
        "TRN2", target_bir_lowering=False, debug=False, num_devices=N_CORES
    )
    bf16 = mybir.dt.bfloat16
    f32 = mybir.dt.float32
    xt_d = nc.dram_tensor("xt", [NM, KT, KP], bf16, kind="ExternalInput")
    wt_d = nc.dram_tensor("wt", [KT, NK * P], bf16, kind="ExternalInput")
    mk_d = nc.dram_tensor("mk", [KT, NK * P], bf16, kind="ExternalInput")
    out_d = nc.dram_tensor("out", [NM, MT, P], f32, kind="ExternalOutput")

    with tile.TileContext(nc) as tc:
        with (
            tc.tile_pool(name="wpool", bufs=1) as wpool,
            tc.tile_pool(name="xpool", bufs=3) as xpool,
            tc.tile_pool(name="opool", bufs=3) as opool,
            tc.tile_pool(name="pspool", bufs=4, space=bass.MemorySpace.PSUM) as pspool,
        ):
            wt_t = wpool.tile([KT, NK * P], bf16)
            mk_t = wpool.tile([KT, NK * P], bf16)
            mw = wpool.tile([KT, NK * P], bf16)
            nc.sync.dma_start(wt_t[:], wt_d[:])
            nc.sync.dma_start(mk_t[:], mk_d[:])
            for k in range(NK):
                sl = slice(k * P, (k + 1) * P)
                nc.vector.tensor_mul(mw[:, sl], wt_t[:, sl], mk_t[:, sl])
            for m in range(NM):
                xt = xpool.tile([KT, KP], bf16)
                nc.sync.dma_start(xt[:], xt_d[m])
                ps = pspool.tile([MT, P], f32)
                for k in range(NK):
                    nc.tensor.matmul(
                        ps[:],
                        xt[:, k * MT : (k + 1) * MT],
                        mw[:, k * P : (k + 1) * P],
                        start=(k == 0),
                        stop=(k == NK - 1),
                    )
                ot = opool.tile([MT, P], f32)
                nc.vector.tensor_copy(ot[:], ps[:])
                nc.sync.dma_start(out_d[m], ot[:])
    nc.compile()
    return nc


def _pack_inputs(x, weight, mask, bias):
    """Host-side shard + pre-tile. Returns in_maps for the 8 cores."""
    xb = np.asarray(x, dtype=np.float32).astype(_BF16)  # (B, G) one cast pass
    wf = np.asarray(weight, dtype=np.float32)
    mf = np.asarray(mask, dtype=np.float32)
    bf = np.asarray(bias, dtype=np.float32)

    in_maps = []
    for core in range(N_CORES):
        g0 = core * GS
        xpad = np.zeros((B, KP), dtype=_BF16)
        xpad[:, :GS] = xb[:, g0 : g0 + GS]
        xpad[:, GS] = _BF16(1.0)  # bias column
        # [m, c, k, p] -> [m, p, k, c]
        xt = np.ascontiguousarray(
            xpad.reshape(NM, MT, NK, KT).transpose(0, 3, 2, 1)
        ).reshape(NM, KT, NK * MT)

        wpad = np.zeros((KP, P), dtype=np.float32)
        wpad[:GS] = wf[:, g0 : g0 + GS].T
        if core == 0:
            wpad[GS] = bf  # bias row (counted exactly once across cores)
        wt = (
            np.ascontiguousarray(wpad.reshape(NK, KT, P).transpose(1, 0, 2))
            .reshape(KT, NK * P)
            .astype(_BF16)
        )

        mpad = np.zeros((KP, P), dtype=np.float32)
        mpad[:GS] = mf[g0 : g0 + GS]
        mpad[GS] = 1.0
        mk = (
            np.ascontiguousarray(mpad.reshape(NK, KT, P).transpose(1, 0, 2))
            .reshape(KT, NK * P)
            .astype(_BF16)
        )
        in_maps.append({"xt": xt, "wt": wt, "mk": mk})
    return in_maps


def kernel(x, weight, mask, bias):
    global LAST_EXEC_TIME_NS, LAST_TRACE, LAST_RESULTS

    profile = bool(int(os.environ.get("KERNEL_PROFILE", "0")))
    if profile:
        _install_profshim()

    nc = _build()
    in_maps = _pack_inputs(x, weight, mask, bias)

    from concourse.bass_utils import run_bass_kernel_spmd

    tmpdir = None
    if profile:
        import tempfile

        base = os.environ.get("KERNEL_TRACE_DIR")
        if base:
            os.makedirs(base, exist_ok=True)
        tmpdir = tempfile.mkdtemp(prefix="ktrace_", dir=base)

    res = run_bass_kernel_spmd(
        nc,
        in_maps,
        core_ids=list(range(N_CORES)),
        trace=profile,
        tmpdir=tmpdir,
    )
    LAST_EXEC_TIME_NS = res.exec_time_ns
    LAST_TRACE = (
        res.instructions_and_trace[1] if res.instructions_and_trace else None
    )
    LAST_RESULTS = res

    parts = np.stack([r["out"].reshape(B, P) for r in res.results])
    return parts.sum(axis=0, dtype=np.float32)
